# revision 70
# baseline (speedup 1.0000x reference)
"""Distributed Trainium2 kernel for nn_Attention_7722351198977.

Math (reference):
    q,k,v = x@Wq, x@Wk, x@Wv          (B,T,H), B=8 T=1024 D=1024 H=64
    s = (q @ k^T) * sqrt(H)           causal mask BEFORE relpos bias
    s = where(tril, s, -inf) + einsum('btc,tvc->btv', q, relpos)
    out = softmax(s) @ v

Sharding: mod-8 interleaved sequence parallelism over query time.
Core i owns queries {8m+i : m=0..127}; local query m attends keys
0..8m+i, padded per query-pair to 16p+16. The causal footprint is
identical on every core (SPMD shape-uniform) while halving relpos
traffic, score compute and softmax work.

Everything flows in fp16 (10 mantissa bits; ~4e-3 end-to-end rel err
validated in numpy) with fp32 PSUM accumulation, so all matmuls
stream at 1 cyc/row instead of fp32's 4.

Per core:
  PASS A: q|k projection (k carries the sqrt(H)=8 scale)
  PASS B: v projection (natural [t, (b,h)] layout)
  two AllGathers (k then v, fp16) on gpsimd -- overlap the bias phase
  bias: causal-packed relpos streamed into SBUF (8 x 1MB DMAs),
    pair-stacked block-diag q matmuls 4-wide col-tiled per quad;
    evict fp16; SBUF->SBUF scatter into [row=(b-half, m)] layout
  attention per (batch-pair bp, chunk): both batches stacked on
    partitions [128 = b0 | b1, kmax]: qk matmul -> +bias+mask (DVE) ->
    reduce_max -> exp (fp16, accum denom) -> PE transposes -> PV ->
    1/denom scale at eviction.
"""

import os as _os

import numpy as np

import concourse.bass as bass
from concourse.bass import _add_dep_helper
import concourse.bacc as bacc
import concourse.mybir as mybir
import concourse.tile as tile
from concourse.bass_utils import run_bass_kernel_spmd
from concourse.masks import make_identity

F32 = mybir.dt.float32
F16 = mybir.dt.float16
B, T, D, H = 8, 1024, 1024, 64
NC = 8            # cores
TC = T // NC      # 128 queries per core
NPAIR = TC // 2   # 64 pairs of local queries
NQUAD = NPAIR // 4
MASK_VAL = -20000.0   # fits fp16; exp(-20000 - max) == 0

# causal extents
KMAX_P = [16 * p + 16 for p in range(NPAIR)]          # per pair
KMAX_Q = [64 * q + 64 for q in range(NQUAD)]          # per quad (4 pairs)
QOFF = [0] * NQUAD                                    # quad col offset in bias_sb
for _q in range(1, NQUAD):
    QOFF[_q] = QOFF[_q - 1] + KMAX_Q[_q - 1]
BIAS_COLS = QOFF[-1] + KMAX_Q[-1]                     # 8704
RELP_OFF = [8 * p * (p + 1) for p in range(NPAIR)]    # pair col offset in relp
RELP_COLS = RELP_OFF[-1] + KMAX_P[-1]                 # 33280

Copy = mybir.ActivationFunctionType.Copy
Exp = mybir.ActivationFunctionType.Exp
DEBUG = _os.environ.get("ATTN_DEBUG", "0") == "1"


def build(num_cores: int = NC) -> bass.Bass:
    nc = bacc.Bacc(
        "TRN2", target_bir_lowering=False, debug=False, num_devices=num_cores
    )

    xT = nc.declare_dram_parameter("xT", [D, B * TC], F16, isOutput=False)
    wqk = nc.declare_dram_parameter("wqk", [D, 2 * H], F16, isOutput=False)
    wv = nc.declare_dram_parameter("wv", [D, H], F16, isOutput=False)
    relp = nc.declare_dram_parameter("relp", [128, RELP_COLS], F16, isOutput=False)
    mask0 = nc.declare_dram_parameter("mask0", [64, 512], F16, isOutput=False)
    mask1 = nc.declare_dram_parameter("mask1", [64, 1024], F16, isOutput=False)
    out_e = nc.declare_dram_parameter("out", [B * TC, H], F32, isOutput=True)

    stg = [
        nc.dram_tensor("stg0", [8, 8, 8, 512], F16),
        nc.dram_tensor("stg1", [8, 8, 8, 1024], F16),
    ]
    cc_in = nc.dram_tensor("cc_in", [TC, B * TC], F16)
    cc_out = nc.dram_tensor("cc_out", [NC * TC, B * TC], F16, addr_space="Shared")
    if DEBUG:
        dbg_q = nc.declare_dram_parameter("dbg_q", [H, B * TC], F16, isOutput=True)
        dbg_k = nc.declare_dram_parameter("dbg_k", [H, B * NC * TC], F16, isOutput=True)
        dbg_v = nc.declare_dram_parameter("dbg_v", [128, 8 * B * H], F16, isOutput=True)
        dbg_bias = nc.declare_dram_parameter("dbg_bias", [128, BIAS_COLS], F16, isOutput=True)
        dbg_tc0 = nc.declare_dram_parameter("dbg_tc0", [64, 8 * 512], F16, isOutput=True)
        dbg_tc1 = nc.declare_dram_parameter("dbg_tc1", [64, 8 * 1024], F16, isOutput=True)

    with tile.TileContext(nc) as tc:
        with (
            tc.tile_pool(name="const", bufs=1) as constp,
            tc.tile_pool(name="big", bufs=1) as bigp,
            tc.tile_pool(name="attn", bufs=2) as attnp,
            tc.tile_pool(name="small", bufs=8) as smallp,
        ):
            # ---- constants / inputs to SBUF ----
            ident = constp.tile([128, 128], F32)
            make_identity(nc, ident[:])
            ident16 = constp.tile([64, 64], F16)
            nc.vector.tensor_copy(ident16[:], ident[0:64, 0:64])

            # weights first (tiny, unblock PASS A), then x in per-c
            # contiguous slices so PASS A starts after the first chunk
            wqk_sb = constp.tile([128, 8, 2 * H], F16)
            nc.scalar.dma_start(
                out=wqk_sb[:], in_=wqk.rearrange("(c p) m -> p c m", p=128)
            )
            wv_sb = constp.tile([128, 8, H], F16)
            nc.scalar.dma_start(
                out=wv_sb[:], in_=wv.rearrange("(c p) m -> p c m", p=128)
            )
            xT_sb = constp.tile([128, 8, B * TC], F16)
            for c in range(8):
                nc.scalar.dma_start(
                    out=xT_sb[:, c, :], in_=xT[c * 128 : (c + 1) * 128, :]
                )
            mask0_sb = constp.tile([64, 512], F16)
            nc.scalar.dma_start(out=mask0_sb[:], in_=mask0[:, :])
            mask1_sb = constp.tile([64, 1024], F16)
            nc.scalar.dma_start(out=mask1_sb[:], in_=mask1[:, :])

            # hoist all gpsimd memsets so the bias phase never waits on them
            qstage = constp.tile([128, NPAIR * 16], F16)
            nc.gpsimd.memset(qstage[:], 0.0)
            bsb = [
                bigp.tile([128, 8, 512], F16, name="bsb0"),
                bigp.tile([128, 8, 1024], F16, name="bsb1"),
            ]
            nc.gpsimd.memset(bsb[0][:], 0.0)
            nc.gpsimd.memset(bsb[1][:], 0.0)
            zero_sb = constp.tile([128, 512], F32)
            nc.gpsimd.memset(zero_sb[:], 0.0)

            # relpos: 8 x ~1MB streaming DMAs, split across both HWDGE rings
            relp_sb = bigp.tile([128, RELP_COLS], F16)
            NRD = 8
            rw = RELP_COLS // NRD
            for r in range(NRD):
                eng = nc.sync if r % 2 == 0 else nc.scalar
                eng.dma_start(
                    out=relp_sb[:, r * rw : (r + 1) * rw],
                    in_=relp[:, r * rw : (r + 1) * rw],
                )

            qT_sb = constp.tile([H, B * TC], F16)
            kT_loc = constp.tile([H, B * TC], F16)
            v_loc = constp.tile([128, B * H], F16)

            with tc.tile_pool(name="psproj", bufs=2, space="PSUM") as ps_pj:
                # ---- PASS A: qT | kT projection ----
                psA = ps_pj.tile([128, 1024], F32, tag="pj", name="psA")
                for h2 in range(2):
                    for c in range(8):
                        nc.tensor.matmul(
                            psA[:, h2 * 512 : (h2 + 1) * 512],
                            wqk_sb[:, c, :],
                            xT_sb[:, c, h2 * 512 : (h2 + 1) * 512],
                            start=(c == 0),
                            stop=(c == 7),
                        )
                nc.scalar.activation(qT_sb[:], psA[0:H, :], Copy)
                # fold the sqrt(H)=8 score scale into k
                nc.scalar.activation(kT_loc[:], psA[H:128, :], Copy, scale=8.0)
                # cc_in on the sync ring: SP idles here anyway, and gpsimd
                # dispatch latency would delay the collective trigger
                h_cck = nc.sync.dma_start(out=cc_in[0:H, :], in_=kT_loc[:])

                # ---- PASS B: v natural ([t_local, (b,h)]) ----
                psV = ps_pj.tile([128, 1024], F32, tag="pj", name="psV")
                for b in range(B):
                    for c in range(8):
                        nc.tensor.matmul(
                            psV[:, b * H : (b + 1) * H],
                            xT_sb[:, c, b * TC : (b + 1) * TC],
                            wv_sb[:, c, :],
                            start=(c == 0),
                            stop=(c == 7),
                        )
                nc.vector.tensor_copy(v_loc[:], psV[:, 0 : B * H])
                h_ccv = nc.sync.dma_start(
                    out=cc_in[H:TC, :].rearrange("p (a c) -> (p a) c", a=2),
                    in_=v_loc[:],
                )
                cc = nc.gpsimd.collective_compute(
                    "AllGather",
                    mybir.AluOpType.bypass,
                    replica_groups=[list(range(num_cores))],
                    ins=[cc_in[:]],
                    outs=[cc_out[:]],
                )
                _add_dep_helper(
                    cc.ins, h_cck.ins, sync=True,
                    reason="collective after cc_in k write",
                )
                _add_dep_helper(
                    cc.ins, h_ccv.ins, sync=True,
                    reason="collective after cc_in v write",
                )

            kT_nat = bigp.tile([H, B, NC, TC], F16)
            v_nat = bigp.tile([128, 8, B, H], F16)
            kT2 = bigp.tile([H, B, T], F16)

            # ---- block-diag q stage for bias matmuls ----
            qsrc = qT_sb.rearrange("c (b pp s) -> c pp s b", b=B, pp=NPAIR, s=2)
            qdst_lo = qstage[0:64, :].rearrange(
                "c (pp s b) -> c pp s b", pp=NPAIR, s=2, b=B
            )
            qdst_hi = qstage[64:128, :].rearrange(
                "c (pp s b) -> c pp s b", pp=NPAIR, s=2, b=B
            )
            nc.vector.tensor_copy(qdst_lo[:, :, 0, :], qsrc[:, :, 0, :])
            nc.vector.tensor_copy(qdst_hi[:, :, 1, :], qsrc[:, :, 1, :])

            # ---- bias phase: pair matmuls, 4-wide col-tiled per quad ----
            # bsb[chunk][32*pm4 + 8*s + b, qd8, v] = bias(pair 4qd+pm4, s, b, v)
            evicts = []
            with tc.tile_pool(name="psbias", bufs=2, space="PSUM") as ps_b:
                for qd in range(NQUAD):
                    kq = KMAX_Q[qd]
                    psQ = ps_b.tile([128, 1024], F32, tag="bq", name=f"psQ{qd}")
                    if qd < 2:
                        # no stale NaNs in PSUM on first use of each buf
                        for sg in range(2):
                            nc.scalar.activation(
                                psQ[:, sg * 512 : (sg + 1) * 512], zero_sb[:], Copy
                            )
                    for pm4 in range(4):
                        p = 4 * qd + pm4
                        kp = KMAX_P[p]
                        for sg in range((kp + 511) // 512):
                            w = min(512, kp - sg * 512)
                            nc.tensor.matmul(
                                psQ[32 * pm4 : 32 * pm4 + 16,
                                    sg * 512 : sg * 512 + w],
                                qstage[:, 16 * p : 16 * p + 16],
                                relp_sb[:, RELP_OFF[p] + sg * 512 :
                                        RELP_OFF[p] + sg * 512 + w],
                                tile_position=(0, 32 * pm4),
                                start=True,
                                stop=True,
                            )
                    # evict all 128 partitions; the unused 16-row strips carry
                    # finite garbage that only ever lands in masked territory
                    if qd < 8:
                        ev = nc.vector.tensor_copy(
                            bsb[0][:, qd, 0:kq], psQ[:, 0:kq]
                        )
                    else:
                        ev = nc.scalar.activation(
                            bsb[1][:, qd - 8, 0:kq], psQ[:, 0:kq], Copy,
                        )
                    evicts.append(ev)

            # ---- scatter bias into attention layout via DRAM staging ----
            # tc[chunk][8*qd8 + 2*pm4 + s, b, v] = bsb[chunk][32*pm4+8*s+b, qd8, v]
            # hop1 moves 8-row groups (pt = 2*pm4+s) to DRAM; hop2 gathers
            # per-quad rows back. All on one ring + explicit completion chain.
            tcx = [
                bigp.tile([64, B, 512], F16, name="tc0"),
                bigp.tile([64, B, 1024], F16, name="tc1"),
            ]
            for chunk in range(2):
                # chunk0 on the sync ring, chunk1 on the scalar ring;
                # stride-2 completion chains keep ordering sound with two
                # transfers in flight per ring
                eng = nc.sync if chunk == 0 else nc.scalar
                hops1 = []
                for pm4 in range(4):
                    # one [16-row, 8-quad] strip per DMA (rows 32*pm4+8s+b
                    # contiguous): 4 hop1s instead of 8, shorter chain
                    h = eng.dma_start(
                        out=stg[chunk][2 * pm4 : 2 * pm4 + 2],
                        in_=bsb[chunk][32 * pm4 : 32 * pm4 + 16],
                    )
                    if len(hops1) >= 2:
                        _add_dep_helper(
                            h.ins, hops1[-2].ins, sync=True,
                            reason="chain scatter hop1 completions",
                        )
                    hops1.append(h)
                hops2 = []
                for qd8 in range(8):
                    h = eng.dma_start(
                        out=tcx[chunk][8 * qd8 : 8 * qd8 + 8],
                        in_=stg[chunk][:, :, qd8, :],
                    )
                    for d in ([hops1[-1], hops1[-2]] if qd8 < 2
                              else [hops2[-2]]):
                        _add_dep_helper(
                            h.ins, d.ins, sync=True,
                            reason="chain scatter hop2 completions",
                        )
                    hops2.append(h)
                # fold the causal mask in (rows are chunk-local m)
                for b in range(B):
                    a = nc.vector.tensor_tensor(
                        out=tcx[chunk][:, b, :], in0=tcx[chunk][:, b, :],
                        in1=(mask0_sb if chunk == 0 else mask1_sb)[:],
                        op=mybir.AluOpType.add,
                    )
                    if b < 2:
                        _add_dep_helper(
                            a.ins, hops2[-1 - b].ins, sync=True,
                            reason="mask add after scatter hop2",
                        )

            if DEBUG:
                nc.sync.dma_start(out=dbg_q[:, :], in_=qT_sb[:])
                nc.sync.dma_start(
                    out=dbg_k.rearrange("r (b j m) -> r b j m", b=B, j=NC),
                    in_=kT_nat[:],
                )
                nc.sync.dma_start(
                    out=dbg_v.rearrange("p (kb b h) -> p kb b h", kb=8, b=B),
                    in_=v_nat[:],
                )
                nc.sync.dma_start(
                    out=dbg_tc0.rearrange("p (b v) -> p b v", b=B),
                    in_=tcx[0][:],
                )
                nc.sync.dma_start(
                    out=dbg_tc1.rearrange("p (b v) -> p b v", b=B),
                    in_=tcx[1][:],
                )

            # readbacks, placed after the scatter in program order so no
            # engine stalls on the collective semaphore with bias-phase
            # work still pending. kT on the idle gpsimd queue; v on sync
            # (HWDGE, faster than serial SWDGE) behind the scatter chain.
            nc.gpsimd.dma_start(
                out=kT_nat[:],
                in_=cc_out.rearrange(
                    "(j hf r) (b m) -> hf r b j m", j=NC, hf=2, r=H, b=B
                )[0],
            )
            # v_nat [p=(ml2,a,j), kb, b, h]: global v = 128*kb + p
            cvv = cc_out.rearrange(
                "(j hf kb ml2) (a b h) -> hf kb ml2 a j b h",
                j=NC, hf=2, kb=8, ml2=8, a=2, b=B, h=H,
            )[1]
            for kb in range(8):
                nc.sync.dma_start(out=v_nat[:, kb, :, :], in_=cvv[kb])

            # reorder kT to v-contiguous [c, b, v] so score matmuls stream a
            # dense rhs. Placed here (after all bias-phase DVE/ACT work in
            # program order) so the engines don't stall on the collective
            # semaphore before finishing the bias phase.
            for b in range(B):
                src = kT_nat[:, b, :, :].rearrange("c j m -> c m j")
                if b % 2 == 0:
                    nc.vector.tensor_copy(kT2[:, b, :], src)
                else:
                    nc.scalar.activation(kT2[:, b, :], src, Copy)

            # ---- attention: software-pipelined, phase2 trails phase1 by
            # DEPTH iterations so PE always has score matmuls queued ahead
            # of the transposes that wait on exp ----
            out_all = constp.tile([64, B, 2, H], F32)
            DEPTH = 3
            attns = {}
            dens = {}
            with (
                tc.tile_pool(name="pss", bufs=DEPTH, space="PSUM") as ps_s,
                tc.tile_pool(name="psatt", bufs=DEPTH + 2) as attp,
                tc.tile_pool(name="pst", bufs=1, space="PSUM") as ps_t,
                tc.tile_pool(name="pso", bufs=1, space="PSUM") as ps_o,
            ):
                # chunk-major order: all chunk0 iterations first -- tc0's
                # scatter chain (sync ring) finishes well before tc1's
                # (scalar ring), so chunk0 softmax overlaps tc1's tail
                SEQ = [(b, 0) for b in range(B)] + [(b, 1) for b in range(B)]
                for step in range(16 + DEPTH):
                    if step < 16:
                        b, chunk = SEQ[step]
                        N = 512 if chunk == 0 else 1024
                        psS = ps_s.tile([64, 1024], F32, tag="s", name="psS")
                        lhs = qT_sb[:, b * TC + 64 * chunk :
                                    b * TC + 64 * chunk + 64]
                        for sg in range(N // 512):
                            nc.tensor.matmul(
                                psS[:, sg * 512 : (sg + 1) * 512],
                                lhs,
                                kT2[:, b, sg * 512 : (sg + 1) * 512],
                                start=True, stop=True,
                            )
                        nc.vector.tensor_tensor(
                            out=psS[:, 0:N], in0=psS[:, 0:N],
                            in1=tcx[chunk][:, b, :],
                            op=mybir.AluOpType.add,
                        )
                        negmax = smallp.tile([64, 1], F32, tag="nm")
                        nc.vector.reduce_max(
                            negmax[:], psS[:, 0:N], axis=mybir.AxisListType.X,
                            negate=True,
                        )
                        attn = attp.tile([64, 1024], F16, tag="a")
                        den = smallp.tile([64, 1], F32, tag="d")
                        nc.scalar.activation(
                            attn[:, 0:N], psS[:, 0:N], Exp,
                            bias=negmax[:], scale=1.0, accum_out=den[:],
                        )
                        attns[step] = attn
                        dens[step] = den
                    if step >= DEPTH:
                        it = step - DEPTH
                        b, chunk = SEQ[it]
                        N = 512 if chunk == 0 else 1024
                        attn = attns.pop(it)
                        den = dens.pop(it)
                        psO = ps_o.tile([64, H], F32, tag="o", name=f"psO{it}")
                        psT = ps_t.tile([128, 512], F16, tag="t", name="psT")
                        for s8 in range(N // 128):
                            nc.tensor.transpose(
                                psT[:, 64 * s8 : 64 * s8 + 64],
                                attn[:, 128 * s8 : 128 * s8 + 128],
                                ident16[:],
                            )
                        aT = attnp.tile([128, 512], F16, tag="aT")
                        nc.scalar.activation(
                            aT[:, 0 : N // 2], psT[:, 0 : N // 2], Copy
                        )
                        nv = N // 128
                        for s8 in range(nv):
                            nc.tensor.matmul(
                                psO[:],
                                aT[:, 64 * s8 : 64 * s8 + 64],
                                v_nat[:, s8, b, :],
                                start=(s8 == 0), stop=(s8 == nv - 1),
                            )
                        rden = smallp.tile([64, 1], F32, tag="r")
                        nc.vector.reciprocal(rden[:], den[:])
                        nc.scalar.activation(
                            out_all[:, b, chunk, :], psO[:], Copy,
                            scale=rden[:],
                        )
            # out_e rows (b, mc*64 + m64)
            nc.sync.dma_start(
                out=out_e.rearrange(
                    "(b mc m64) h -> m64 b mc h", b=B, mc=2, m64=64
                ),
                in_=out_all[:],
            )
    nc.compile()
    return nc


_CACHE: dict = {}


def _get_nc():
    if "nc" not in _CACHE:
        _CACHE["nc"] = build(NC)
    return _CACHE["nc"]


def _prep_inputs(x, Wq, Wk, Wv, relpos):
    x = np.asarray(x, dtype=np.float32)
    relpos = np.asarray(relpos, dtype=np.float32)
    wqk = np.ascontiguousarray(
        np.concatenate([np.asarray(Wq), np.asarray(Wk)], axis=1)
    ).astype(np.float16)
    wv = np.ascontiguousarray(np.asarray(Wv)).astype(np.float16)
    in_maps = []
    for i in range(NC):
        tq = 8 * np.arange(TC) + i                     # global query ids
        xs = x[:, tq, :]                               # (B, TC, D)
        xT = np.ascontiguousarray(
            xs.transpose(2, 0, 1).reshape(D, B * TC)
        ).astype(np.float16)
        # causal-packed relpos: pair p rows = [R_t0^T ; R_t1^T], kmax_p cols
        rp = np.empty((128, RELP_COLS), dtype=np.float16)
        for p in range(NPAIR):
            kp = KMAX_P[p]
            t0, t1 = tq[2 * p], tq[2 * p + 1]
            rp[0:64, RELP_OFF[p] : RELP_OFF[p] + kp] = (
                relpos[t0, 0:kp, :].T.astype(np.float16)
            )
            rp[64:128, RELP_OFF[p] : RELP_OFF[p] + kp] = (
                relpos[t1, 0:kp, :].T.astype(np.float16)
            )
        # masks in attention layout: row = chunk-local m
        vv0 = np.arange(512)[None, :]
        vv1 = np.arange(1024)[None, :]
        m0 = tq[0:64][:, None]                         # chunk0 queries
        m1 = tq[64:128][:, None]                       # chunk1 queries
        msk0 = np.where(vv0 <= m0, 0.0, MASK_VAL).astype(np.float16)
        msk1 = np.where(vv1 <= m1, 0.0, MASK_VAL).astype(np.float16)
        in_maps.append(
            {"xT": xT, "wqk": wqk, "wv": wv, "relp": rp,
             "mask0": np.ascontiguousarray(msk0),
             "mask1": np.ascontiguousarray(msk1)}
        )
    return in_maps


def run_sharded(in_maps, trace=False, **kw):
    nc = _get_nc()
    return run_bass_kernel_spmd(
        nc, in_maps, core_ids=list(range(NC)), trace=trace, **kw
    )


def kernel(x, Wq, Wk, Wv, relpos):
    in_maps = _prep_inputs(x, Wq, Wk, Wv, relpos)
    res = run_sharded(in_maps, trace=False)
    out = np.empty((B, T, H), dtype=np.float32)
    for i in range(NC):
        tq = 8 * np.arange(TC) + i
        out[:, tq, :] = res.results[i]["out"].reshape(B, TC, H)
    return out
